# revision 23
# baseline (speedup 1.0000x reference)
"""Mixed-precision quantized linear (fp32/int8/int4/int2 weight groups) on 8 trn2 cores.

Strategy: tensor-parallel over output channels. Core k owns rows
[k*n_g/8, (k+1)*n_g/8) of every bit-group (128 + 384 + 512 + 256 = 1280
channels). x is replicated. Host pre-transposes weights to [K, N] layout
(K on partitions), with K globally permuted to evens-then-odds so that the
packed 4-bit nibbles (low=even K, high=odd K) unpack into contiguous K tiles.

v2: w16 and q8 are staged as one bf16 tensor (`wq`, 512 channels) so they
need no on-device dequant (int8 values are exact in bf16); the 4/2-bit
groups stay packed and unpack with DVE bit-ops + ACT casts. The per-channel
scale arrives pre-broadcast ([128, 1280] f32); the bias row is folded in as
a K=1 matmul. Matmuls run in two K-phases (kt 0-15, 16-31) so PE can start
after half the wq DMA. Each core writes a [256, 1280] slice; host scatters
the 8 slices into the final [256, 11008] via the idx arrays.
"""

import numpy as np
import ml_dtypes

import concourse.bass as bass
import concourse.bacc as bacc
import concourse.mybir as mybir
import concourse.tile as tile
from concourse.bass_utils import run_bass_kernel_spmd

IN = 4096
OUT = 11008
N16, N8, N4, N2 = 1024, 3072, 4096, 2048
M = 256
NCORES = 8
C16, C8, C4, C2 = N16 // 8, N8 // 8, N4 // 8, N2 // 8  # 128, 384, 512, 256
NCH = C16 + C8 + C4 + C2  # 1280
KT = IN // 128  # 32 K-tiles of 128
KP = KT // 2  # 16 packed K-tiles (nibbles)

WQW = KT * (C16 + C8)  # 16384 (bf16 w16+q8, 512 channels)
P4W = KP * C4  # 8192 packed bytes
P2W = KP * C2  # 4096

BF16 = mybir.dt.bfloat16
F32 = mybir.dt.float32
I8 = mybir.dt.int8

Alu = mybir.AluOpType
Act = mybir.ActivationFunctionType

ACT_CAST = True  # use the scalar (ACT) engine for the dequant cast ops


def _build_nc(loop_n=1, act_cast=None):
    if act_cast is None:
        act_cast = ACT_CAST
    nc = bacc.Bacc()
    xt_d = nc.declare_dram_parameter("xt", [128, KT * M], BF16, isOutput=False)
    wq_d = nc.declare_dram_parameter("wq", [128, WQW], BF16, isOutput=False)
    pp_d = nc.declare_dram_parameter("pp", [128, P4W + P2W], I8, isOutput=False)
    sbc_d = nc.declare_dram_parameter("sbc", [128, NCH], F32, isOutput=False)
    brow_d = nc.declare_dram_parameter("brow", [1, NCH], BF16, isOutput=False)
    out_d = nc.declare_dram_parameter("out", [M, NCH], F32, isOutput=True)

    with tile.TileContext(nc) as tc:
        with (
            tc.tile_pool(name="big", bufs=1) as pool,
            tc.tile_pool(name="tmp", bufs=4) as tpool,
            tc.tile_pool(name="psum", bufs=1, space="PSUM") as ppool,
        ):
            xs = pool.tile([128, KT * M], BF16)
            wqs = pool.tile([128, WQW], BF16)
            pps = pool.tile([128, P4W + P2W], I8)
            p4d = pool.tile([128, KT * C4], BF16)
            p2d = pool.tile([128, KT * C2], BF16)
            sbcs = pool.tile([128, NCH], F32)
            brs = pool.tile([1, NCH], BF16)
            brs2 = pool.tile([1, NCH], BF16)
            ones_b = pool.tile([1, 128], BF16)
            outs = pool.tile([128, 2 * NCH], F32)

            p4s = pps[:, :P4W]
            p2s = pps[:, P4W:]

            import contextlib

            loop_ctx = (
                tc.For_i(0, loop_n, 1, hint_engines=mybir.ALL_ENGINES)
                if loop_n > 1
                else contextlib.nullcontext()
            )
            with loop_ctx:
                # ---- input DMAs, ordered by consumer urgency:
                # bias row (tiny) -> packed nibbles (dequant is the long pole)
                # -> x -> wq in halves (PE phase 0 starts after half) -> scale
                # broadcast (needed only at the first epilogue)
                half = WQW // 2
                nc.sync.dma_start(out=brs[:], in_=brow_d[:])
                nc.sync.dma_start(out=pps[:], in_=pp_d[:])
                nc.sync.dma_start(out=xs[:], in_=xt_d[:])
                nc.sync.dma_start(out=wqs[:, :half], in_=wq_d[:, :half])
                nc.sync.dma_start(out=wqs[:, half:], in_=wq_d[:, half:])
                nc.sync.dma_start(out=sbcs[:], in_=sbc_d[:])

                nc.vector.memset(ones_b[:], 1.0)
                # bounce the bias row through DVE so the K=1 bias matmuls
                # have all-DVE deps (matmul carries only one sem wait)
                nc.vector.tensor_copy(brs2[:], brs[:])

                # ---- unpack 4-bit groups, low nibbles first (PE phase order)
                # low = ((b & 15) ^ 8) - 8 ; high = (b & 0xF0) * (1/16)
                def deq(kind, src, dst, cw, kp):
                    b = src[:, kp * cw : (kp + 1) * cw]
                    if kind == "lo":
                        d = dst[:, kp * cw : (kp + 1) * cw]
                        t = tpool.tile([128, C4], I8, tag="deq")
                        nc.vector.tensor_scalar(
                            t[:, :cw], b, 15, 8,
                            op0=Alu.bitwise_and, op1=Alu.bitwise_xor,
                        )
                        if act_cast:
                            nc.scalar.activation(
                                d, t[:, :cw], Act.Copy, bias=-8.0, scale=1.0
                            )
                        else:
                            nc.vector.tensor_scalar(
                                d, t[:, :cw], 8, None, op0=Alu.subtract
                            )
                    else:
                        d = dst[:, (kp + KP) * cw : (kp + KP + 1) * cw]
                        t = tpool.tile([128, C4], I8, tag="deq2")
                        nc.vector.tensor_scalar(
                            t[:, :cw], b, -16, None, op0=Alu.bitwise_and
                        )
                        if act_cast:
                            nc.scalar.activation(
                                d, t[:, :cw], Act.Copy, bias=0.0, scale=0.0625
                            )
                        else:
                            nc.vector.tensor_scalar(
                                d, t[:, :cw], 0.0625, None, op0=Alu.mult
                            )

                for kind in ("lo", "hi"):
                    for src, dst, cw in ((p4s, p4d, C4), (p2s, p2d, C2)):
                        for kp in range(KP):
                            deq(kind, src, dst, cw, kp)

                # ---- main GEMMs in two K-phases per block
                chunks = [(0, C16 + C8, wqs), (512, C4, p4d), (1024, C2, p2d)]
                for blk in range(2):
                    ps = [
                        ppool.tile([128, 512], F32, name=f"ps_{blk}_{ci}", tag=f"ps_{blk}_{ci}")
                        for ci in range(len(chunks))
                    ]
                    for phase in range(2):
                        for ci, (c0, cw, w) in enumerate(chunks):
                            for kt in range(phase * 16, phase * 16 + 16):
                                nc.tensor.matmul(
                                    ps[ci][:, :cw],
                                    xs[:, kt * M + blk * 128 : kt * M + blk * 128 + 128],
                                    w[:, kt * cw : (kt + 1) * cw],
                                    start=(kt == 0),
                                    stop=False,
                                    skip_group_check=True,
                                )
                    for ci, (c0, cw, w) in enumerate(chunks):
                        nc.tensor.matmul(
                            ps[ci][:, :cw], ones_b[:1, :], brs2[:1, c0 : c0 + cw],
                            start=False, stop=True, skip_group_check=True,
                        )
                        nc.vector.scalar_tensor_tensor(
                            outs[:, blk * NCH + c0 : blk * NCH + c0 + cw],
                            ps[ci][:, :cw], 1.0, sbcs[:, c0 : c0 + cw],
                            op0=Alu.mult, op1=Alu.mult,
                        )
                    # per-block output DMA overlaps block 1 compute
                    out_v = out_d[:].rearrange("(b p) n -> p b n", p=128)
                    nc.sync.dma_start(
                        out=out_v[:, blk, :],
                        in_=outs[:, blk * NCH : (blk + 1) * NCH],
                    )
    nc.finalize()
    return nc


def _tile128(a):
    """[K, F] -> [128, (K//128)*F] so DRAM layout matches the SBUF tile."""
    k, f = a.shape
    t = k // 128
    return np.ascontiguousarray(
        a.reshape(t, 128, f).transpose(1, 0, 2).reshape(128, t * f)
    )


_CACHE = {}


def stage_inputs(**inputs):
    x = np.asarray(inputs["x"], dtype=np.float32)
    w16 = np.asarray(inputs["w16"], dtype=np.float32)
    b16 = np.asarray(inputs["b16"], dtype=np.float32)
    q8 = np.asarray(inputs["q8"])
    s8 = np.asarray(inputs["s8"], dtype=np.float32)
    b8 = np.asarray(inputs["b8"], dtype=np.float32)
    p4 = np.asarray(inputs["p4"])
    s4 = np.asarray(inputs["s4"], dtype=np.float32)
    b4 = np.asarray(inputs["b4"], dtype=np.float32)
    p2 = np.asarray(inputs["p2"])
    s2 = np.asarray(inputs["s2"], dtype=np.float32)
    b2 = np.asarray(inputs["b2"], dtype=np.float32)
    idx16 = np.asarray(inputs["idx16"])
    idx8 = np.asarray(inputs["idx8"])
    idx4 = np.asarray(inputs["idx4"])
    idx2 = np.asarray(inputs["idx2"])

    bf16 = ml_dtypes.bfloat16
    permK = np.concatenate([np.arange(0, IN, 2), np.arange(1, IN, 2)])

    xt = _tile128(np.ascontiguousarray(x.T[permK]).astype(bf16))

    in_maps = []
    for k in range(NCORES):
        w16k = w16[k * C16 : (k + 1) * C16]
        q8k = q8[k * C8 : (k + 1) * C8]
        p4k = p4[k * C4 : (k + 1) * C4]
        p2k = p2[k * C2 : (k + 1) * C2]
        s8k = s8[k * C8 : (k + 1) * C8, 0]
        s4k = s4[k * C4 : (k + 1) * C4, 0]
        s2k = s2[k * C2 : (k + 1) * C2, 0]
        b16k = b16[k * C16 : (k + 1) * C16]
        b8k = b8[k * C8 : (k + 1) * C8]
        b4k = b4[k * C4 : (k + 1) * C4]
        b2k = b2[k * C2 : (k + 1) * C2]

        # bf16 [4096, 512] = [w16 | q8] in permuted-K row order
        wqT = np.concatenate(
            [w16k.T, q8k.astype(np.float32).T], axis=1
        )[permK].astype(bf16)
        wq = _tile128(np.ascontiguousarray(wqT))
        pp = np.concatenate(
            [
                _tile128(np.ascontiguousarray(p4k.astype(np.int8).T)),
                _tile128(np.ascontiguousarray(p2k.astype(np.int8).T)),
            ],
            axis=1,
        )
        srow = np.concatenate([np.ones(C16, np.float32), s8k, s4k, s2k])
        sbc = np.ascontiguousarray(
            np.broadcast_to(srow[None, :], (128, NCH))
        ).astype(np.float32)
        brow = (
            np.concatenate([b16k, b8k / s8k, b4k / s4k, b2k / s2k])
            .reshape(1, NCH)
            .astype(bf16)
        )

        in_maps.append({"xt": xt, "wq": wq, "pp": pp, "sbc": sbc, "brow": brow})

    cat_idxs = [
        np.concatenate(
            [
                idx16[k * C16 : (k + 1) * C16],
                idx8[k * C8 : (k + 1) * C8],
                idx4[k * C4 : (k + 1) * C4],
                idx2[k * C2 : (k + 1) * C2],
            ]
        )
        for k in range(NCORES)
    ]
    return in_maps, cat_idxs


def kernel(**inputs):
    in_maps, cat_idxs = stage_inputs(**inputs)
    if "nc" not in _CACHE:
        _CACHE["nc"] = _build_nc()
    res = run_bass_kernel_spmd(_CACHE["nc"], in_maps, core_ids=list(range(NCORES)))
    _CACHE["last_res"] = res

    out = np.zeros((M, OUT), dtype=np.float32)
    for k in range(NCORES):
        out[:, cat_idxs[k]] = res.results[k]["out"]
    return out


# revision 26
# speedup vs baseline: 4.2171x; 4.2171x over previous
"""Mixed-precision quantized linear (fp32/int8/int4/int2 weight groups) on 8 trn2 cores.

Strategy: tensor-parallel over output channels. Core k owns rows
[k*n_g/8, (k+1)*n_g/8) of every bit-group (128 + 384 + 512 + 256 = 1280
channels). x is replicated. Host pre-transposes weights to [K, N] layout
(K on partitions), with K globally permuted to evens-then-odds so that the
packed 4-bit nibbles (low=even K, high=odd K) unpack into contiguous K tiles.

v2: w16 and q8 are staged as one bf16 tensor (`wq`, 512 channels) so they
need no on-device dequant (int8 values are exact in bf16); the 4/2-bit
groups stay packed and unpack with DVE bit-ops + ACT casts. The per-channel
scale arrives pre-broadcast ([128, 1280] f32); the bias row is folded in as
a K=1 matmul. Matmuls run in two K-phases (kt 0-15, 16-31) so PE can start
after half the wq DMA. Each core writes a [256, 1280] slice; host scatters
the 8 slices into the final [256, 11008] via the idx arrays.
"""

import numpy as np
import ml_dtypes

import concourse.bass as bass
import concourse.bacc as bacc
import concourse.mybir as mybir
import concourse.tile as tile
from concourse.bass_utils import run_bass_kernel_spmd

IN = 4096
OUT = 11008
N16, N8, N4, N2 = 1024, 3072, 4096, 2048
M = 256
NCORES = 8
C16, C8, C4, C2 = N16 // 8, N8 // 8, N4 // 8, N2 // 8  # 128, 384, 512, 256
NCH = C16 + C8 + C4 + C2  # 1280
KT = IN // 128  # 32 K-tiles of 128
KP = KT // 2  # 16 packed K-tiles (nibbles)

WQW = KT * (C16 + C8)  # 16384 (bf16 w16+q8, 512 channels)
P4W = KP * C4  # 8192 packed bytes
P2W = KP * C2  # 4096

BF16 = mybir.dt.bfloat16
F32 = mybir.dt.float32
I8 = mybir.dt.int8

Alu = mybir.AluOpType
Act = mybir.ActivationFunctionType

ACT_CAST = False  # ACT int8 casts are ~7x slower on real HW than the cost model claims


def _build_nc(loop_n=1, act_cast=None, chunk_inner=False):
    if act_cast is None:
        act_cast = ACT_CAST
    nc = bacc.Bacc()
    xt_d = nc.declare_dram_parameter("xt", [128, KT * M], BF16, isOutput=False)
    wq_d = nc.declare_dram_parameter("wq", [128, WQW], BF16, isOutput=False)
    pp_d = nc.declare_dram_parameter("pp", [128, P4W + P2W], I8, isOutput=False)
    sbc_d = nc.declare_dram_parameter("sbc", [128, NCH], F32, isOutput=False)
    brow_d = nc.declare_dram_parameter("brow", [1, NCH], BF16, isOutput=False)
    out_d = nc.declare_dram_parameter("out", [M, NCH], F32, isOutput=True)

    with tile.TileContext(nc) as tc:
        with (
            tc.tile_pool(name="big", bufs=1) as pool,
            tc.tile_pool(name="tmp", bufs=4) as tpool,
            tc.tile_pool(name="psum", bufs=1, space="PSUM") as ppool,
        ):
            xs = pool.tile([128, KT * M], BF16)
            wqs = pool.tile([128, WQW], BF16)
            pps = pool.tile([128, P4W + P2W], I8)
            p4d = pool.tile([128, KT * C4], BF16)
            p2d = pool.tile([128, KT * C2], BF16)
            sbcs = pool.tile([128, NCH], F32)
            brs = pool.tile([1, NCH], BF16)
            brs2 = pool.tile([1, NCH], BF16)
            ones_b = pool.tile([1, 128], BF16)
            outs = pool.tile([128, 2 * NCH], F32)

            p4s = pps[:, :P4W]
            p2s = pps[:, P4W:]

            import contextlib

            loop_ctx = (
                tc.For_i(0, loop_n, 1, hint_engines=mybir.ALL_ENGINES)
                if loop_n > 1
                else contextlib.nullcontext()
            )
            with loop_ctx:
                # ---- input DMAs, ordered by consumer urgency:
                # bias row (tiny) -> packed nibbles (dequant is the long pole)
                # -> x -> wq in halves (PE phase 0 starts after half) -> scale
                # broadcast (needed only at the first epilogue)
                half = WQW // 2
                xh = KT * M // 2
                nc.sync.dma_start(out=brs[:], in_=brow_d[:])
                nc.sync.dma_start(out=pps[:], in_=pp_d[:])
                nc.sync.dma_start(out=xs[:, :xh], in_=xt_d[:, :xh])
                nc.sync.dma_start(out=wqs[:, :half], in_=wq_d[:, :half])
                nc.sync.dma_start(out=wqs[:, half:], in_=wq_d[:, half:])
                nc.sync.dma_start(out=xs[:, xh:], in_=xt_d[:, xh:])
                nc.sync.dma_start(out=sbcs[:], in_=sbc_d[:])

                nc.vector.memset(ones_b[:], 1.0)
                # bounce the bias row through DVE so the K=1 bias matmuls
                # have all-DVE deps (matmul carries only one sem wait)
                nc.vector.tensor_copy(brs2[:], brs[:])

                # ---- unpack 4-bit groups, low nibbles first (PE phase order)
                # low = ((b & 15) ^ 8) - 8 ; high = (b & 0xF0) * (1/16)
                def deq(kind, src, dst, cw, kp):
                    b = src[:, kp * cw : (kp + 1) * cw]
                    if kind == "lo":
                        d = dst[:, kp * cw : (kp + 1) * cw]
                        t = tpool.tile([128, C4], I8, tag="deq")
                        nc.vector.tensor_scalar(
                            t[:, :cw], b, 15, 8,
                            op0=Alu.bitwise_and, op1=Alu.bitwise_xor,
                        )
                        if act_cast:
                            nc.scalar.activation(
                                d, t[:, :cw], Act.Copy, bias=-8.0, scale=1.0
                            )
                        else:
                            nc.vector.tensor_scalar(
                                d, t[:, :cw], 8, None, op0=Alu.subtract
                            )
                    else:
                        d = dst[:, (kp + KP) * cw : (kp + KP + 1) * cw]
                        t = tpool.tile([128, C4], I8, tag="deq2")
                        nc.vector.tensor_scalar(
                            t[:, :cw], b, -16, None, op0=Alu.bitwise_and
                        )
                        if act_cast:
                            nc.scalar.activation(
                                d, t[:, :cw], Act.Copy, bias=0.0, scale=0.0625
                            )
                        else:
                            nc.vector.tensor_scalar(
                                d, t[:, :cw], 0.0625, None, op0=Alu.mult
                            )

                for kind in ("lo", "hi"):
                    for src, dst, cw in ((p4s, p4d, C4), (p2s, p2d, C2)):
                        for kp in range(KP):
                            deq(kind, src, dst, cw, kp)

                # ---- main GEMMs in two K-phases per block
                chunks = [(0, C16 + C8, wqs), (512, C4, p4d), (1024, C2, p2d)]
                for blk in range(2):
                    ps = [
                        ppool.tile([128, 512], F32, name=f"ps_{blk}_{ci}", tag=f"ps_{blk}_{ci}")
                        for ci in range(len(chunks))
                    ]
                    for phase in range(2):
                        if chunk_inner:
                            iters = [
                                (ci, c)
                                for _ in (0,)
                                for ci, c in enumerate(chunks)
                            ]
                            for kt in range(phase * 16, phase * 16 + 16):
                                for ci, (c0, cw, w) in enumerate(chunks):
                                    nc.tensor.matmul(
                                        ps[ci][:, :cw],
                                        xs[:, blk * (KT * 128) + kt * 128 : blk * (KT * 128) + kt * 128 + 128],
                                        w[:, kt * cw : (kt + 1) * cw],
                                        start=(kt == 0),
                                        stop=False,
                                        skip_group_check=True,
                                    )
                        else:
                            for ci, (c0, cw, w) in enumerate(chunks):
                                for kt in range(phase * 16, phase * 16 + 16):
                                    nc.tensor.matmul(
                                        ps[ci][:, :cw],
                                        xs[:, blk * (KT * 128) + kt * 128 : blk * (KT * 128) + kt * 128 + 128],
                                        w[:, kt * cw : (kt + 1) * cw],
                                        start=(kt == 0),
                                        stop=False,
                                        skip_group_check=True,
                                    )
                    for ci, (c0, cw, w) in enumerate(chunks):
                        nc.tensor.matmul(
                            ps[ci][:, :cw], ones_b[:1, :], brs2[:1, c0 : c0 + cw],
                            start=False, stop=True, skip_group_check=True,
                        )
                        nc.vector.scalar_tensor_tensor(
                            outs[:, blk * NCH + c0 : blk * NCH + c0 + cw],
                            ps[ci][:, :cw], 1.0, sbcs[:, c0 : c0 + cw],
                            op0=Alu.mult, op1=Alu.mult,
                        )
                    # per-block output DMA overlaps block 1 compute
                    out_v = out_d[:].rearrange("(b p) n -> p b n", p=128)
                    nc.sync.dma_start(
                        out=out_v[:, blk, :],
                        in_=outs[:, blk * NCH : (blk + 1) * NCH],
                    )
    nc.finalize()
    return nc


def _tile128(a):
    """[K, F] -> [128, (K//128)*F] so DRAM layout matches the SBUF tile."""
    k, f = a.shape
    t = k // 128
    return np.ascontiguousarray(
        a.reshape(t, 128, f).transpose(1, 0, 2).reshape(128, t * f)
    )


_CACHE = {}


def stage_inputs(**inputs):
    x = np.asarray(inputs["x"], dtype=np.float32)
    w16 = np.asarray(inputs["w16"], dtype=np.float32)
    b16 = np.asarray(inputs["b16"], dtype=np.float32)
    q8 = np.asarray(inputs["q8"])
    s8 = np.asarray(inputs["s8"], dtype=np.float32)
    b8 = np.asarray(inputs["b8"], dtype=np.float32)
    p4 = np.asarray(inputs["p4"])
    s4 = np.asarray(inputs["s4"], dtype=np.float32)
    b4 = np.asarray(inputs["b4"], dtype=np.float32)
    p2 = np.asarray(inputs["p2"])
    s2 = np.asarray(inputs["s2"], dtype=np.float32)
    b2 = np.asarray(inputs["b2"], dtype=np.float32)
    idx16 = np.asarray(inputs["idx16"])
    idx8 = np.asarray(inputs["idx8"])
    idx4 = np.asarray(inputs["idx4"])
    idx2 = np.asarray(inputs["idx2"])

    bf16 = ml_dtypes.bfloat16
    permK = np.concatenate([np.arange(0, IN, 2), np.arange(1, IN, 2)])

    xTp = np.ascontiguousarray(x.T[permK]).astype(bf16)  # [4096, 256]
    # block-major tiling: [128, blk*(KT*128) + kt*128 + tok]
    t = xTp.reshape(KT, 128, 2, 128).transpose(2, 0, 1, 3)  # [blk, kt, p, tok]
    xt = np.ascontiguousarray(t.transpose(2, 0, 1, 3).reshape(128, 2 * KT * 128))

    in_maps = []
    for k in range(NCORES):
        w16k = w16[k * C16 : (k + 1) * C16]
        q8k = q8[k * C8 : (k + 1) * C8]
        p4k = p4[k * C4 : (k + 1) * C4]
        p2k = p2[k * C2 : (k + 1) * C2]
        s8k = s8[k * C8 : (k + 1) * C8, 0]
        s4k = s4[k * C4 : (k + 1) * C4, 0]
        s2k = s2[k * C2 : (k + 1) * C2, 0]
        b16k = b16[k * C16 : (k + 1) * C16]
        b8k = b8[k * C8 : (k + 1) * C8]
        b4k = b4[k * C4 : (k + 1) * C4]
        b2k = b2[k * C2 : (k + 1) * C2]

        # bf16 [4096, 512] = [w16 | q8] in permuted-K row order
        wqT = np.concatenate(
            [w16k.T, q8k.astype(np.float32).T], axis=1
        )[permK].astype(bf16)
        wq = _tile128(np.ascontiguousarray(wqT))
        pp = np.concatenate(
            [
                _tile128(np.ascontiguousarray(p4k.astype(np.int8).T)),
                _tile128(np.ascontiguousarray(p2k.astype(np.int8).T)),
            ],
            axis=1,
        )
        srow = np.concatenate([np.ones(C16, np.float32), s8k, s4k, s2k])
        sbc = np.ascontiguousarray(
            np.broadcast_to(srow[None, :], (128, NCH))
        ).astype(np.float32)
        brow = (
            np.concatenate([b16k, b8k / s8k, b4k / s4k, b2k / s2k])
            .reshape(1, NCH)
            .astype(bf16)
        )

        in_maps.append({"xt": xt, "wq": wq, "pp": pp, "sbc": sbc, "brow": brow})

    cat_idxs = [
        np.concatenate(
            [
                idx16[k * C16 : (k + 1) * C16],
                idx8[k * C8 : (k + 1) * C8],
                idx4[k * C4 : (k + 1) * C4],
                idx2[k * C2 : (k + 1) * C2],
            ]
        )
        for k in range(NCORES)
    ]
    return in_maps, cat_idxs


def kernel(**inputs):
    in_maps, cat_idxs = stage_inputs(**inputs)
    if "nc" not in _CACHE:
        _CACHE["nc"] = _build_nc()
    res = run_bass_kernel_spmd(_CACHE["nc"], in_maps, core_ids=list(range(NCORES)))
    _CACHE["last_res"] = res

    out = np.zeros((M, OUT), dtype=np.float32)
    for k in range(NCORES):
        out[:, cat_idxs[k]] = res.results[k]["out"]
    return out
